# revision 1
# baseline (speedup 1.0000x reference)
"""Trainium2 Bass kernel for the 4-kernel MMD permutation test (nn_DUAL_78237124264373).

Math (per core, 25 of the 200 permutations; everything else replicated):
  Z = [X; Y] (768 x 64), d2[r,c] = ||Z_r - Z_c||^2 built on the PE as a single
  rank-66 matmul  d2 = L^T R  with L = [Zt; sq; 1], R = [-2 Zt; 1; sq].
  K0_k = f_k(d2) (symmetric kernel matrix, no diag zeroing).
  With a_p the X-half indicator of permutation p and the zeroed-K statistics
  expressed through symmetric-K0 quantities plus corrections through
  e_j = K0[j, 384+j] (the zeroed stripe), every U_b entry reduces to
     U_b = kap*(q0 - arow0) + W_corr @ e_k + (2/c2)*t + C_k
  where q0 = a K0 a, arow0 = a K0 1 come from one matmul M0 = A_aug K0,
  t is the per-permutation paired-sample sum computed from host-arranged
  Z-row pairs (sentinel rows handle zeroed-stripe pairs), and W_corr folds
  the three correction coefficients into one host-built matrix.

Layout: the four kernels are column-tiled onto PE col-groups, so all
per-permutation statistics live at partition 32*k + p (kernel k, perm p) and
the DVE reductions run once over 128 partitions instead of 4x over 27.
"""

import os
import sys

import numpy as np

if "/opt/trn_rl_repo" not in sys.path:
    sys.path.insert(0, "/opt/trn_rl_repo")

import concourse.bacc as bacc
import concourse.bass as bass
import concourse.mybir as mybir
import concourse.tile as tile
from concourse import bass_utils

N = 384
NM = 768
D = 64
NPER = 200
NC = 8
PPC = NPER // NC  # 25
C1 = float(N * (N - 1))
C2 = float(N * N)
KAP = np.float32(2.0 / C1 + 2.0 / C2)
CB1 = np.float32(1.0 / C1 + 2.0 / C2)
CB2 = np.float32(1.0 / C1)
TCO = np.float32(2.0 / C2)
IC1 = np.float32(1.0 / C1)
IC2 = np.float32(1.0 / C2)
KERNELS = ("gaussian", "laplacian", "gaussian", "laplacian")

F32 = mybir.dt.float32
F32R = mybir.dt.float32r
BF16 = mybir.dt.bfloat16
AF = mybir.ActivationFunctionType
ALU = mybir.AluOpType


def _build():
    nc = bacc.Bacc("TRN2", target_bir_lowering=False, debug=False)
    with tile.TileContext(nc) as tc:
        with tc.tile_pool(name="dram", bufs=1, space="DRAM") as dram, \
             tc.tile_pool(name="io", bufs=1) as io, \
             tc.tile_pool(name="big", bufs=1) as big, \
             tc.tile_pool(name="kpool", bufs=4) as kpool, \
             tc.tile_pool(name="scr", bufs=2) as scr, \
             tc.tile_pool(name="sml", bufs=1) as sml:

            def din(name, shape, dt=F32):
                return dram.tile(shape, dt, kind="ExternalInput", name=name,
                                 uniquify=False)

            # One fused input tensor (single DMA); column layout below.
            W_IN = 2002
            bigin_d = din("bigin", [128, W_IN])
            zp_d = din("zp", [128, 9984 + 192], BF16)  # pair rows + bf16 atp
            out_d = dram.tile([4, 1 + PPC], F32, kind="ExternalOutput",
                              name="out", uniquify=False)

            # ---- phase 0: input DMAs (Zt block first: it gates the PE) ----
            bigin = io.tile([128, W_IN], F32, name="bigin_sb")
            nc.sync.dma_start(out=bigin[:, 0:NM], in_=bigin_d[:, 0:NM])
            nc.sync.dma_start(out=bigin[:, NM:], in_=bigin_d[:, NM:])
            Lbig = bigin[0:D + 1, 0:NM]          # [Zt rows 0-63; ones row 64]
            astk = bigin[:, NM:2 * NM]           # A_aug rows at 32k+i
            atp = bigin[:, 1536:1536 + 192]      # A_aug^T chunks (32-padded)
            wct = bigin[:, 1728:1728 + 96]       # W_corr^T chunks (32-padded)
            fold = bigin[0:75, 1824:1824 + 32]   # 3->1 fold (32-padded)
            ident = bigin[:, 1856:1856 + 128]
            aux = bigin[:, 1984:1994]
            aux4 = bigin[0:1, 1994:2002]
            zpf = io.tile([128, 78 * 128 + 192], BF16, name="zp_sb")
            nc.sync.dma_start(out=zpf[:], in_=zp_d[:])
            zp = zpf[:, 0:9984].rearrange("p (b d) -> p b d", d=128)
            atpb = zpf[:, 9984:9984 + 192]       # A_aug^T chunks in bf16

            ones = io.tile([128, 1], F32, name="ones_sb")
            nc.vector.memset(ones[:], 1.0)

            R_all = io.tile([D + 1, NM], F32, name="R_all")
            # cols 0:4608 = the 6 row-tiles of d2; cols 4608:4686 = the 78
            # pair-distance columns, so ONE wide sqrt covers both.
            d2sb = big.tile([128, 6 * NM + 78], F32, name="d2sb")
            dist = big.tile([128, 6 * NM + 78], F32, name="dist_sb")
            M0sb = big.tile([128, NM], F32, name="M0sb")

            with tc.tile_pool(name="psA", bufs=3, space="PSUM") as psA:
                # ---- phase 1: sq = rowsums of Zt^2, landed at psum
                # partitions 0 (for the sq_col transposes) and 64 (for the
                # R matrix row) via col-tiling ----
                zt2 = scr.tile([D, NM], F32, name="zt2", tag="zt2", bufs=1)
                nc.vector.tensor_tensor(out=zt2[:], in0=Lbig[0:D, :],
                                        in1=Lbig[0:D, :], op=ALU.mult)
                ps_sq = psA.tile([128, NM], F32, name="ps_sq", tag="d2")
                for s in (slice(0, 512), slice(512, NM)):
                    nc.tensor.matmul(ps_sq[0:1, s], ones[0:D, 0:1], zt2[:, s],
                                     start=True, stop=True,
                                     skip_group_check=True)
                    nc.tensor.matmul(ps_sq[D:D + 1, s], ones[0:D, 0:1],
                                     zt2[:, s], start=True, stop=True,
                                     tile_position=(0, D),
                                     skip_group_check=True)
                sqrow = sml.tile([1, NM], F32, name="sqrow")
                nc.vector.tensor_copy(sqrow[:], ps_sq[0:1, :])
                # R = [-2 Zt; sq]; row 64 copies within partition 64.
                nc.vector.tensor_scalar_mul(R_all[0:D, :], Lbig[0:D, :], -2.0)
                nc.vector.tensor_copy(R_all[D:D + 1, :], ps_sq[D:D + 1, :])
                # sq as columns (for the relu bias): 6 tiny PE transposes
                ps_sqc = psA.tile([128, 8], F32, name="ps_sqc", tag="sqc",
                                  bufs=1)
                for r in range(6):
                    nc.tensor.matmul(ps_sqc[:, r:r + 1],
                                     sqrow[0:1, 128 * r:128 * (r + 1)],
                                     ones[0:1, 0:1], is_transpose=True,
                                     start=True, stop=True,
                                     skip_group_check=True)
                sqc = sml.tile([128, 8], F32, name="sqc")
                nc.vector.tensor_copy(sqc[:, 0:6], ps_sqc[:, 0:6])
                # per-gaussian fused bias: (sq[r] + 1e-12) * scale_k
                sqsc = {}
                for k in (0, 2):
                    t = sml.tile([128, 8], F32, name=f"sqsc{k}")
                    nc.vector.tensor_scalar(
                        out=t[:, 0:6], in0=sqc[:, 0:6],
                        scalar1=aux[:, 2 * k:2 * k + 1],
                        scalar2=aux[:, 2 * k + 1:2 * k + 2],
                        op0=ALU.mult, op1=ALU.add)
                    sqsc[k] = t

                kts = [kpool.tile([128, 6 * NM], BF16, name=f"kt{k}",
                                  tag="kt") for k in range(4)]

                # ---- phase 2: psum = -2 Z Z^T + sq[c].  Per row-tile: the
                # DVE adds sq[r] and clamps into d2sb while the two gaussian
                # kernels exp straight out of PSUM (exp of the tiny negative
                # diagonal values is harmless). The DVE also squeezes the
                # bf16 pair-distance pieces into its matmul-wait gaps. ----
                pdiff = sml.tile([128, 78, 64], BF16, name="pdiff")
                pprod = sml.tile([128, 78, 64], BF16, name="pprod")
                for r in range(6):
                    ps_d2 = psA.tile([128, NM], F32, name=f"ps_d2_{r}",
                                     tag="d2")
                    lhs = Lbig[:, 128 * r:128 * (r + 1)]
                    nc.tensor.matmul(ps_d2[:, 0:512], lhs, R_all[:, 0:512],
                                     start=True, stop=True)
                    nc.tensor.matmul(ps_d2[:, 512:NM], lhs, R_all[:, 512:NM],
                                     start=True, stop=True)
                    sl = slice(NM * r, NM * (r + 1))
                    nc.vector.tensor_scalar(
                        out=d2sb[:, sl], in0=ps_d2[:],
                        scalar1=sqc[:, r:r + 1], scalar2=0.0,
                        op0=ALU.add, op1=ALU.max)
                    for k in (0, 2):
                        nc.scalar.activation(kts[k][:, sl], ps_d2[:], AF.Exp,
                                             scale=aux[:, 2 * k:2 * k + 1],
                                             bias=sqsc[k][:, r:r + 1])
                    j = r if r < 3 else r - 3
                    js = slice(26 * j, 26 * (j + 1))
                    if r < 3:
                        nc.vector.tensor_tensor(out=pdiff[:, js, :],
                                                in0=zp[:, js, 0:64],
                                                in1=zp[:, js, 64:128],
                                                op=ALU.subtract)
                    else:
                        nc.vector.tensor_tensor(out=pprod[:, js, :],
                                                in0=pdiff[:, js, :],
                                                in1=pdiff[:, js, :],
                                                op=ALU.mult)
                for j in range(3):
                    js = slice(26 * j, 26 * (j + 1))
                    nc.vector.tensor_reduce(
                        d2sb[:, 6 * NM + 26 * j:6 * NM + 26 * (j + 1)],
                        pprod[:, js, :], axis=mybir.AxisListType.X,
                        op=ALU.add)

            # ---- phase 4: dist = sqrt(d2 + 1e-12), pair cols included ----
            nc.scalar.activation(dist[:], d2sb[:], AF.Sqrt, bias=aux[:, 8:9])
            distp = dist[:, 6 * NM:6 * NM + 78]

            arow = sml.tile([128, 1], F32, name="arow")
            colA = sml.tile([128, 1], F32, name="colA")
            q0c = sml.tile([128, 1], F32, name="q0c")

            with tc.tile_pool(name="psB", bufs=1, space="PSUM") as psB, \
                 tc.tile_pool(name="psC", bufs=1, space="PSUM") as psC:
                # ---- phase 5: laplacian K tiles; M0 = A_aug K0 col-tiled so
                # kernel k's rows land at partitions 32k+i ----
                ps_m = psB.tile([128, NM], F32, name="ps_m")
                for k in (1, 3):
                    for h in range(2):
                        hs = slice(3 * NM * h, 3 * NM * (h + 1))
                        nc.scalar.activation(kts[k][:, hs], dist[:, hs],
                                             AF.Exp,
                                             scale=aux[:, 2 * k:2 * k + 1],
                                             bias=aux[:, 2 * k + 1:2 * k + 2])
                for c in range(6):
                    lhs = atpb[:, 32 * c:32 * (c + 1)]
                    for k in range(4):
                        pr = slice(32 * k, 32 * k + 32)
                        nc.tensor.matmul(ps_m[pr, 0:512], lhs,
                                         kts[k][:, NM * c:NM * c + 512],
                                         start=(c == 0), stop=(c == 5),
                                         tile_position=(0, 32 * k),
                                         skip_group_check=True)
                        nc.tensor.matmul(ps_m[pr, 512:NM], lhs,
                                         kts[k][:, NM * c + 512:NM * (c + 1)],
                                         start=(c == 0), stop=(c == 5),
                                         tile_position=(0, 32 * k),
                                         skip_group_check=True)
                # row stats: copy+rowsum fused, first-half sum, masked q0
                nc.vector.tensor_scalar(
                    out=M0sb[:], in0=ps_m[:], scalar1=1.0, scalar2=0.0,
                    op0=ALU.mult, op1=ALU.add, accum_out=arow[:])
                sA = scr.tile([128, N], F32, name="sA", tag="sA")
                nc.vector.tensor_scalar(
                    out=sA[:], in0=M0sb[:, 0:N], scalar1=1.0, scalar2=0.0,
                    op0=ALU.mult, op1=ALU.add, accum_out=colA[:])
                sB = scr.tile([128, NM], F32, name="sB", tag="sB")
                nc.vector.tensor_tensor(out=sB[:], in0=M0sb[:], in1=astk[:],
                                        op=ALU.mult)
                nc.vector.tensor_reduce(q0c[:], sB[:],
                                        axis=mybir.AxisListType.X, op=ALU.add)

                # ---- pair-term exps (Exp table is already loaded) ----
                # t_k via column-sum matmul then a fold matmul into
                # partitions 32k+p
                d2p = d2sb[:, 6 * NM:6 * NM + 78]
                ps_t = psC.tile([75, 4], F32, name="ps_t", tag="sm", bufs=3)
                expks = []
                for k, kern in enumerate(KERNELS):
                    psrc = d2p if kern == "gaussian" else distp
                    expk = scr.tile([128, 78], F32, name=f"expk{k}",
                                    tag="expk", bufs=4)
                    nc.scalar.activation(expk[:], psrc, AF.Exp,
                                         scale=aux[:, 2 * k:2 * k + 1],
                                         bias=aux[:, 2 * k + 1:2 * k + 2])
                    expks.append(expk)
                    nc.tensor.matmul(ps_t[:, k:k + 1], expk[:, 0:75],
                                     ones[:, 0:1], start=True, stop=True)
                t75s = sml.tile([75, 4], F32, name="t75s")
                nc.vector.tensor_copy(t75s[:], ps_t[:])
                ps_tc = psC.tile([128, 1], F32, name="ps_tc", tag="sm", bufs=3)
                for k in range(4):
                    nc.tensor.matmul(ps_tc[32 * k:32 * k + 32, 0:1], fold[:],
                                     t75s[:, k:k + 1], start=True, stop=True,
                                     tile_position=(0, 32 * k),
                                     skip_group_check=True)
                tcol = sml.tile([128, 1], F32, name="tcol")
                nc.vector.tensor_scalar_mul(tcol[:], ps_tc[:], float(TCO))

                # ---- phase 6: corrections (col-tiled) and stripe sums ----
                ps_corr = psC.tile([128, 1], F32, name="ps_corr", tag="sm",
                                   bufs=3)
                for c in range(3):
                    for k in range(4):
                        nc.tensor.matmul(
                            ps_corr[32 * k:32 * k + 32, 0:1],
                            wct[:, 32 * c:32 * (c + 1)],
                            expks[k][:, 75 + c:76 + c],
                            start=(c == 0), stop=(c == 2),
                            tile_position=(0, 32 * k),
                            skip_group_check=True)
                sesum = sml.tile([3, 4], F32, name="sesum")
                for k in range(4):
                    ps_sek = psC.tile([3, 1], F32, name=f"ps_se{k}", tag="se",
                                      bufs=2)
                    nc.tensor.matmul(ps_sek[:], expks[k][:, 75:78],
                                     ones[:, 0:1], start=True, stop=True)
                    nc.vector.tensor_copy(sesum[:, k:k + 1], ps_sek[:])

                # ---- phase 7: U_b assembly in the stacked [128,1] layout ----
                colB = sml.tile([128, 1], F32, name="colB")
                nc.vector.tensor_tensor(out=colB[:], in0=arow[:], in1=colA[:],
                                        op=ALU.subtract)
                ubv = sml.tile([128, 1], F32, name="ubv")
                nc.vector.tensor_tensor(out=ubv[:], in0=q0c[:], in1=arow[:],
                                        op=ALU.subtract)
                nc.vector.tensor_scalar_mul(ubv[:], ubv[:], float(KAP))
                nc.vector.tensor_tensor(out=ubv[:], in0=ubv[:], in1=ps_corr[:],
                                        op=ALU.add)
                nc.vector.tensor_tensor(out=ubv[:], in0=ubv[:], in1=tcol[:],
                                        op=ALU.add)
                # ---- phase 8: fold everything into one partition-0 row ----
                # frow: [0:128)=ub, [128:256)=colA^T, [256:384)=colB^T,
                # [384:396)=sesum
                frow = sml.tile([1, 396], F32, name="frow")
                nc.sync.dma_start(out=frow[0:1, 0:128], in_=ubv[:])
                nc.sync.dma_start(out=frow[0:1, 128:256], in_=colA[:])
                nc.sync.dma_start(out=frow[0:1, 256:384], in_=colB[:])
                nc.sync.dma_start(out=frow[0:1, 384:396], in_=sesum[:])

                def fr(base, step=32, count=4):
                    ap = frow[0:1, base:base + 1]
                    return bass.AP(ap.tensor, ap.offset,
                                   [ap.ap[0], [step, count]])

                XXv = fr(128 + 25)
                YXv = fr(128 + 26)
                XY0v = fr(256 + 25)
                YYv = fr(256 + 26)
                # se_k = sum_c sesum[4c+k]
                sev = sml.tile([1, 4], F32, name="sev")
                nc.vector.tensor_reduce(
                    sev[:],
                    frow[0:1, 384:396].rearrange("o (c k) -> o k c", k=4),
                    axis=mybir.AxisListType.X, op=ALU.add)
                s0t = sml.tile([1, 4], F32, name="s0t")
                nc.vector.tensor_tensor(out=s0t[:], in0=XXv, in1=YXv,
                                        op=ALU.add)
                nc.vector.tensor_tensor(out=s0t[:], in0=s0t[:], in1=XY0v,
                                        op=ALU.add)
                nc.vector.tensor_tensor(out=s0t[:], in0=s0t[:], in1=YYv,
                                        op=ALU.add)
                ck = sml.tile([1, 4], F32, name="ck")
                nc.vector.tensor_tensor(out=ck[:], in0=s0t[:], in1=sev[:],
                                        op=ALU.subtract)
                nc.vector.tensor_tensor(out=ck[:], in0=ck[:],
                                        in1=aux4[0:1, 0:4], op=ALU.subtract)
                nc.vector.tensor_scalar_mul(ck[:], ck[:], float(IC1))
                u1 = sml.tile([1, 4], F32, name="u1")
                nc.vector.tensor_tensor(out=u1[:], in0=XXv, in1=YYv,
                                        op=ALU.add)
                nc.vector.tensor_tensor(out=u1[:], in0=u1[:],
                                        in1=aux4[0:1, 0:4], op=ALU.subtract)
                nc.vector.tensor_scalar_mul(u1[:], u1[:], float(IC1))
                u2 = sml.tile([1, 4], F32, name="u2")
                nc.vector.tensor_tensor(out=u2[:], in0=XY0v, in1=sev[:],
                                        op=ALU.subtract)
                nc.vector.tensor_scalar_mul(u2[:], u2[:], float(2.0 * IC2))

                # ---- phase 9: contiguous U row + U_b block, two out DMAs ----
                uF = sml.tile([1, 4], F32, name="uF")
                nc.vector.tensor_tensor(out=uF[:], in0=u1[:], in1=u2[:],
                                        op=ALU.subtract)
                ubc = sml.tile([1, 4 * PPC], F32, name="ubc")
                ub_src = frow[0:1, 0:128].rearrange("o (k p) -> o k p", p=32)
                ckap = ck[0:1, 0:4]
                ck_b = bass.AP(ckap.tensor, ckap.offset,
                               [ckap.ap[0], [1, 4], [0, PPC]])
                nc.vector.tensor_tensor(
                    out=ubc[0:1, :].rearrange("o (k p) -> o k p", p=PPC),
                    in0=ub_src[0:1, :, 0:PPC], in1=ck_b, op=ALU.add)
                nc.sync.dma_start(
                    out=out_d[:, 0:1],
                    in_=uF[0:1, :].rearrange("o (k w) -> o k w", w=1))
                nc.sync.dma_start(
                    out=out_d[:, 1:1 + PPC],
                    in_=ubc[0:1, :].rearrange("o (k p) -> o k p", p=PPC))

    nc.compile()
    return nc


def _host_prep(X, Y, bandwidths, perms):
    X = np.ascontiguousarray(X, np.float32)
    Y = np.ascontiguousarray(Y, np.float32)
    perms = np.ascontiguousarray(perms, np.int32)
    Zt = np.zeros((D + 1, NM), np.float32)  # rows 0-63 Zt, row 64 ones
    Zt[0:D] = np.concatenate([X, Y], 0).T
    Zt[D] = 1.0
    b = np.asarray(bandwidths, np.float64)
    gs = (-1.0 / (b * b)).astype(np.float32)
    gb = (gs.astype(np.float64) * 1e-12).astype(np.float32)
    ls = (-1.0 / b).astype(np.float32)
    aux = np.zeros((128, 10), np.float32)
    aux[:, 8] = 1e-12
    d0c = np.zeros(4, np.float64)
    for k, kern in enumerate(KERNELS):
        if kern == "gaussian":
            aux[:, 2 * k] = gs[k]
            aux[:, 2 * k + 1] = gb[k]
            d0c[k] = np.exp(-1e-12 / (b[k] * b[k]))
        else:
            aux[:, 2 * k] = ls[k]
            aux[:, 2 * k + 1] = 0.0
            d0c[k] = np.exp(-np.sqrt(1e-12) / b[k])
    aux4 = np.zeros((1, 8), np.float32)
    aux4[0, 0:4] = (768.0 * d0c).astype(np.float32)
    ident = np.eye(128, dtype=np.float32)
    foldm = np.zeros((75, 32), np.float32)
    foldm[:, :PPC] = (np.arange(75)[:, None] // 3 ==
                      np.arange(PPC)[None, :])

    maps = []
    for cid in range(NC):
        pm = perms[cid * PPC:(cid + 1) * PPC]
        A = np.zeros((27, NM), np.float32)
        A[np.arange(PPC)[:, None], pm[:, :N]] = 1
        A[25, :N] = 1
        A[26, N:] = 1
        astk = np.zeros((128, NM), np.float32)
        for k in range(4):
            astk[32 * k:32 * k + 27] = A
        atp = np.zeros((128, 6 * 32), np.float32)
        for c in range(6):
            atp[:, 32 * c:32 * c + 27] = A[:, 128 * c:128 * (c + 1)].T
        A1 = A[:PPC, :N]
        A2 = A[:PPC, N:]
        Wc = (-KAP * (A1 * A2) + CB1 * A1 + CB2 * A2).astype(np.float32)
        wct = np.zeros((128, 3 * 32), np.float32)
        for c in range(3):
            wct[:, 32 * c:32 * c + PPC] = Wc[:, 128 * c:128 * (c + 1)].T
        pX = pm[:, :N].astype(np.int64).ravel()
        pY = pm[:, N:].astype(np.int64).ravel()
        # Pair-arranged Z rows: [zx | zy] per pair; stripe pairs (pY==pX+384)
        # get a sentinel row with huge distance so f_k -> 0 (matches the
        # zeroed K stripe). Rows 9600..9983 are the stripe-diagonal pairs
        # (they produce the e_k correction vectors).
        Zf = np.concatenate([X, Y], 0)
        zx = Zf[pX]
        zy = Zf[pY]
        stripe = pY == pX + N
        zx[stripe] = 0.0
        zy[stripe] = 0.0
        zx[stripe, 0] = 1e6  # d2=1e12: exp(-1e12/b^2)=exp(-1e6/b)=0
        j = np.arange(N)
        zp = np.concatenate([
            np.concatenate([zx, zy], 1),
            np.concatenate([Zf[j], Zf[N + j]], 1),
        ], 0)
        import ml_dtypes
        zp = zp.reshape(78, 128, 128).transpose(1, 0, 2).reshape(128, 9984)
        zp = np.concatenate([zp, atp], 1).astype(ml_dtypes.bfloat16)
        bigin = np.zeros((128, 2002), np.float32)
        bigin[0:D + 1, 0:NM] = Zt
        bigin[:, NM:2 * NM] = astk
        bigin[:, 1536:1536 + 192] = atp
        bigin[:, 1728:1728 + 96] = wct
        bigin[0:75, 1824:1824 + 32] = foldm
        bigin[:, 1856:1856 + 128] = ident
        bigin[:, 1984:1994] = aux
        bigin[0:1, 1994:2002] = aux4
        maps.append(dict(bigin=bigin, zp=zp))
    return maps


_NC_CACHE = None


def _get_nc():
    global _NC_CACHE
    if _NC_CACHE is None:
        _NC_CACHE = _build()
    return _NC_CACHE


def kernel(X, Y, bandwidths, perms):
    nc = _get_nc()
    in_maps = _host_prep(X, Y, bandwidths, perms)
    res = bass_utils.run_bass_kernel_spmd(nc, in_maps, list(range(NC)))
    full = np.zeros((4, 1 + NPER), np.float32)
    full[:, 0] = res.results[0]["out"][:, 0]
    for cid in range(NC):
        full[:, 1 + cid * PPC:1 + (cid + 1) * PPC] = \
            res.results[cid]["out"][:, 1:]
    return full

